# revision 3
# baseline (speedup 1.0000x reference)
"""GNN message-passing layer (Net3DLayer) on 8 Trainium2 NeuronCores.

Strategy:
  * Host: stable-sort edges by dst; split into 8 contiguous dst ranges
    (6250 nodes each) with roughly E/8 edges. Each core fully owns the
    segment-sum + node update for its node range -> no collectives.
  * Within a core, edges are grouped into 128-node "windows"; each
    window's edge run is padded to a multiple of 128 edges so that the
    device program (one SPMD program for all 8 cores) has a fixed,
    data-independent-per-core structure (window subtile counts are
    compile-time constants computed from the actual inputs).
  * Host pre-gathers feat[src], feat[dst] and pre-transposes all edge
    streams to feature-major [128, E_pad] so the device streams them
    with dense DMA (contraction dim on partitions).
  * Device per 512-edge group: message MLP (3+1 matmuls, SiLU on ACT),
    edge gate (matmul + sigmoid + K=1 broadcast matmul + DVE multiply),
    d_out = d + message on GPSIMD, then per 128-edge subtile a PE
    transpose to edge-major and a one-hot scatter matmul
    (m_sum^T[:, win] += m_em^T @ Sel) accumulating in a window PSUM
    tile. When a window completes: node update MLP + residual, store.
"""

import math
from contextlib import ExitStack

import numpy as np

import concourse.bass as bass
import concourse.tile as tile
from concourse import bacc, mybir
from concourse.bass_utils import run_bass_kernel_spmd

N_NODES = 50000
N_EDGES = 800000
HID = 128
P = 128
N_CORES = 8
NPC = N_NODES // N_CORES  # 6250 nodes per core
W_FULL = (NPC + P - 1) // P  # 49 windows per core (last is 106 wide)

F32 = mybir.dt.float32
AF = mybir.ActivationFunctionType
OP = mybir.AluOpType


def _plan(dst_sorted):
    """Window/padding layout shared by all cores (max over cores per slot)."""
    bounds = np.empty((N_CORES, W_FULL + 1), dtype=np.int64)
    for c in range(N_CORES):
        for w in range(W_FULL + 1):
            bounds[c, w] = c * NPC + min(P * w, NPC)
    rs = np.searchsorted(dst_sorted, bounds.reshape(-1)).reshape(N_CORES, W_FULL + 1)
    runlen = rs[:, 1:] - rs[:, :-1]  # [8, 49]
    Lw = np.maximum(1, (-(-runlen // P)).max(axis=0)).astype(np.int64)  # [49]
    S = int(Lw.sum())
    G = (S + 3) // 4
    S_pad = G * 4
    return rs, Lw, S, G, S_pad


def _build_program(G, Lw, S_pad, E_pad, bg_val):
    """Build + compile the SPMD bass program (same for all 8 cores)."""
    nc = bacc.Bacc("TRN2", target_bir_lowering=False, debug=False,
                   num_devices=N_CORES)

    din = {}
    for name, shape in [
        ("fsrcT", [P, E_pad]), ("fdstT", [P, E_pad]), ("dT", [P, E_pad]),
        ("dstl", [G * P, 4]), ("featTs", [P, W_FULL * P]),
        ("w1m", [P, 3 * P]), ("w2m", [P, P]), ("wg", [P, 1]),
        ("w1u", [P, P]), ("w2u", [P, P]),
        ("b1m", [P, 1]), ("b2m", [P, 1]), ("b1u", [P, 1]), ("b2u", [P, 1]),
        ("iota", [P, P]), ("ident", [P, P]), ("ones", [1, P]),
    ]:
        din[name] = nc.dram_tensor(name, shape, F32, kind="ExternalInput").ap()
    d_outT = nc.dram_tensor("d_outT", [P, E_pad], F32, kind="ExternalOutput").ap()
    feat_outT = nc.dram_tensor("feat_outT", [P, W_FULL * P], F32,
                               kind="ExternalOutput").ap()

    # subtile -> (window, position) map; dummy tail window index W_FULL
    w_of, pos_of, len_of = [], [], []
    for w in range(W_FULL):
        for t in range(int(Lw[w])):
            w_of.append(w); pos_of.append(t); len_of.append(int(Lw[w]))
    n_tail = S_pad - len(w_of)
    for t in range(n_tail):
        w_of.append(W_FULL); pos_of.append(t); len_of.append(n_tail)

    with tile.TileContext(nc) as tc, ExitStack() as ctx:
        cpool = ctx.enter_context(tc.tile_pool(name="consts", bufs=1))
        spool = ctx.enter_context(tc.tile_pool(name="stream", bufs=4))
        wpool = ctx.enter_context(tc.tile_pool(name="work", bufs=3))
        epool = ctx.enter_context(tc.tile_pool(name="em", bufs=6))
        npool = ctx.enter_context(tc.tile_pool(name="node", bufs=2))
        pmm = ctx.enter_context(tc.tile_pool(name="pmm", bufs=2, space="PSUM"))
        pgl = ctx.enter_context(tc.tile_pool(name="pgl", bufs=2, space="PSUM"))
        ptr = ctx.enter_context(tc.tile_pool(name="ptr", bufs=2, space="PSUM"))
        pwin = ctx.enter_context(tc.tile_pool(name="pwin", bufs=2, space="PSUM"))

        cst = {}
        for name, shape in [
            ("w1m", [P, 3 * P]), ("w2m", [P, P]), ("wg", [P, 1]),
            ("w1u", [P, P]), ("w2u", [P, P]), ("b1m", [P, 1]), ("b2m", [P, 1]),
            ("b1u", [P, 1]), ("b2u", [P, 1]), ("iota", [P, P]),
            ("ident", [P, P]), ("ones", [1, P]),
        ]:
            t = cpool.tile(shape, F32, name=f"c_{name}", tag=name)
            nc.sync.dma_start(t[:], din[name][:])
            cst[name] = t

        win_tiles = {}

        def node_update(w):
            wt = win_tiles.pop(w)
            fTw = npool.tile([P, P], F32, name="fTw", tag="fTw")
            nc.sync.dma_start(fTw[:], din["featTs"][:, w * P:(w + 1) * P])
            xT = npool.tile([P, P], F32, name="xT", tag="xT")
            nc.vector.tensor_add(xT[:], wt[:], fTw[:])
            ups = ptr.tile([P, P], F32, name="ups", tag="tr")
            nc.tensor.matmul(ups[:], cst["w1u"][:], xT[:], start=True, stop=True)
            uT = npool.tile([P, P], F32, name="uT", tag="uT")
            nc.scalar.activation(uT[:], ups[:], AF.Silu, bias=cst["b1u"][:])
            ops_ = ptr.tile([P, P], F32, name="ops", tag="tr")
            nc.tensor.matmul(ops_[:], cst["w2u"][:], uT[:], start=True, stop=True)
            oT = npool.tile([P, P], F32, name="oT", tag="oT")
            nc.vector.scalar_tensor_tensor(oT[:], ops_[:], cst["b2u"][:], fTw[:],
                                           op0=OP.add, op1=OP.add)
            nc.sync.dma_start(feat_outT[:, w * P:(w + 1) * P], oT[:])

        for g in range(G):
            sl = slice(g * 512, (g + 1) * 512)
            fs = spool.tile([P, 512], F32, name="fs", tag="fs")
            nc.sync.dma_start(fs[:], din["fsrcT"][:, sl])
            fd = spool.tile([P, 512], F32, name="fd", tag="fd")
            nc.sync.dma_start(fd[:], din["fdstT"][:, sl])
            dd = spool.tile([P, 512], F32, name="dd", tag="dd")
            nc.sync.dma_start(dd[:], din["dT"][:, sl])
            dl = spool.tile([P, 4], F32, name="dl", tag="dl")
            nc.sync.dma_start(dl[:], din["dstl"][g * P:(g + 1) * P, :])

            hps = pmm.tile([P, 512], F32, name="hps", tag="mm")
            nc.tensor.matmul(hps[:], cst["w1m"][:, 0:P], fs[:],
                             start=True, stop=False, skip_group_check=True)
            nc.tensor.matmul(hps[:], cst["w1m"][:, P:2 * P], fd[:],
                             start=False, stop=False, skip_group_check=True)
            nc.tensor.matmul(hps[:], cst["w1m"][:, 2 * P:3 * P], dd[:],
                             start=False, stop=True, skip_group_check=True)
            hT = wpool.tile([P, 512], F32, name="hT", tag="hT")
            nc.scalar.activation(hT[:], hps[:], AF.Silu, bias=cst["b1m"][:])

            mps = pmm.tile([P, 512], F32, name="mps", tag="mm")
            nc.tensor.matmul(mps[:], cst["w2m"][:], hT[:], start=True, stop=True,
                             skip_group_check=True)
            msgT = wpool.tile([P, 512], F32, name="msgT", tag="msgT")
            nc.scalar.activation(msgT[:], mps[:], AF.Silu, bias=cst["b2m"][:])

            dout = wpool.tile([P, 512], F32, name="dout", tag="dout")
            nc.gpsimd.tensor_add(dout[:], dd[:], msgT[:])
            nc.sync.dma_start(d_outT[:, sl], dout[:])

            glps = pgl.tile([1, 512], F32, name="glps", tag="gl")
            nc.tensor.matmul(glps[:], cst["wg"][:], msgT[:], start=True,
                             stop=True, skip_group_check=True)
            sgl = wpool.tile([1, 512], F32, name="sgl", tag="sgl")
            nc.scalar.activation(sgl[:], glps[:], AF.Sigmoid, bias=float(bg_val))
            gbps = pmm.tile([P, 512], F32, name="gbps", tag="mm")
            nc.tensor.matmul(gbps[:], cst["ones"][:], sgl[:], start=True,
                             stop=True, skip_group_check=True)
            mT = wpool.tile([P, 512], F32, name="mT", tag="mT")
            nc.vector.tensor_tensor(mT[:], msgT[:], gbps[:], op=OP.mult)

            for j in range(4):
                s = g * 4 + j
                w, pos, lw = w_of[s], pos_of[s], len_of[s]
                tps = ptr.tile([P, P], F32, name="tps", tag="tr")
                nc.tensor.transpose(tps[:], mT[:, j * P:(j + 1) * P],
                                    cst["ident"][:])
                mem = epool.tile([P, P], F32, name="mem", tag="mem")
                if j % 2 == 0:
                    nc.scalar.copy(mem[:], tps[:])
                else:
                    nc.vector.tensor_copy(mem[:], tps[:])
                sel = epool.tile([P, P], F32, name="sel", tag="sel")
                nc.vector.tensor_scalar(sel[:], cst["iota"][:], dl[:, j:j + 1],
                                        None, OP.is_equal)
                if pos == 0:
                    win_tiles[w] = pwin.tile([P, P], F32, name=f"win{w}",
                                             tag="win")
                nc.tensor.matmul(win_tiles[w][:], mem[:], sel[:],
                                 start=(pos == 0), stop=(pos == lw - 1),
                                 skip_group_check=True)
                if pos == lw - 1:
                    if w < W_FULL:
                        node_update(w)
                    else:
                        win_tiles.pop(w)

    nc.compile()
    return nc


def _prepare(feat, d, src, dst, W1m, b1m, W2m, b2m, Wg, bg, W1u, b1u, W2u, b2u):
    feat = np.ascontiguousarray(np.asarray(feat, dtype=np.float32))
    d = np.ascontiguousarray(np.asarray(d, dtype=np.float32))
    src = np.asarray(src).astype(np.int64).ravel()
    dst = np.asarray(dst).astype(np.int64).ravel()
    E = dst.shape[0]

    order = np.argsort(dst, kind="stable")
    dst_s = dst[order]
    rs, Lw, S, G, S_pad = _plan(dst_s)
    E_pad = S_pad * P

    woff = np.concatenate([[0], np.cumsum(Lw)]).astype(np.int64)

    # per-core original-edge-index map (-1 = padding) and local dst offsets
    map_pad = np.full((N_CORES, E_pad), -1, dtype=np.int64)
    dstl_pad = np.full((N_CORES, E_pad), -5.0, dtype=np.float32)
    for c in range(N_CORES):
        for w in range(W_FULL):
            a, b = int(rs[c, w]), int(rs[c, w + 1])
            n = b - a
            if n == 0:
                continue
            base = int(woff[w]) * P
            map_pad[c, base:base + n] = order[a:b]
            dstl_pad[c, base:base + n] = (
                dst_s[a:b] - (c * NPC + P * w)).astype(np.float32)

    featT_padded = np.zeros((P, N_CORES * NPC + P), dtype=np.float32)
    featT_padded[:, :N_NODES] = feat.T

    d_ext = np.vstack([d, np.zeros((1, HID), dtype=np.float32)])

    in_maps = []
    consts = {
        "w1m": np.ascontiguousarray(
            np.concatenate([W1m[0:P], W1m[P:2 * P], W1m[2 * P:3 * P]], axis=1)
        ).astype(np.float32),
        "w2m": np.ascontiguousarray(np.asarray(W2m, np.float32)),
        "wg": np.ascontiguousarray(np.asarray(Wg, np.float32).reshape(P, 1)),
        "w1u": np.ascontiguousarray(np.asarray(W1u, np.float32)),
        "w2u": np.ascontiguousarray(np.asarray(W2u, np.float32)),
        "b1m": np.asarray(b1m, np.float32).reshape(P, 1).copy(),
        "b2m": np.asarray(b2m, np.float32).reshape(P, 1).copy(),
        "b1u": np.asarray(b1u, np.float32).reshape(P, 1).copy(),
        "b2u": np.asarray(b2u, np.float32).reshape(P, 1).copy(),
        "iota": np.broadcast_to(np.arange(P, dtype=np.float32), (P, P)).copy(),
        "ident": np.eye(P, dtype=np.float32),
        "ones": np.ones((1, P), dtype=np.float32),
    }
    for c in range(N_CORES):
        idx = map_pad[c]
        valid = idx >= 0
        safe = np.where(valid, idx, 0)
        src_idx = np.where(valid, src[safe], 0)
        dst_idx = np.where(valid, dst[safe], 0)
        didx = np.where(valid, idx, E)
        m = {
            "fsrcT": np.ascontiguousarray(feat[src_idx].T),
            "fdstT": np.ascontiguousarray(feat[dst_idx].T),
            "dT": np.ascontiguousarray(d_ext[didx].T),
            "dstl": np.ascontiguousarray(
                dstl_pad[c].reshape(G, 4, P).transpose(0, 2, 1).reshape(G * P, 4)
            ),
            "featTs": np.ascontiguousarray(
                featT_padded[:, c * NPC:c * NPC + W_FULL * P]),
        }
        m.update(consts)
        in_maps.append(m)

    bg_val = float(np.asarray(bg, np.float32).ravel()[0])
    meta = dict(E=E, G=G, Lw=Lw, S_pad=S_pad, E_pad=E_pad, map_pad=map_pad,
                bg_val=bg_val)
    return in_maps, meta


def _assemble(results, meta):
    E = meta["E"]
    feat_out = np.empty((N_NODES, HID), dtype=np.float32)
    d_out = np.empty((E, HID), dtype=np.float32)
    for c in range(N_CORES):
        fo = results[c]["feat_outT"]  # [128, W_FULL*128]
        feat_out[c * NPC:(c + 1) * NPC] = fo.T[:NPC]
        do = results[c]["d_outT"]  # [128, E_pad]
        idx = meta["map_pad"][c]
        v = idx >= 0
        d_out[idx[v]] = do.T[v]
    return feat_out, d_out


def run(inputs, trace=False, tmpdir=None):
    in_maps, meta = _prepare(**inputs)
    nc = _build_program(meta["G"], meta["Lw"], meta["S_pad"], meta["E_pad"],
                        meta["bg_val"])
    res = run_bass_kernel_spmd(nc, in_maps, core_ids=list(range(N_CORES)),
                               trace=trace, tmpdir=tmpdir)
    feat_out, d_out = _assemble(res.results, meta)
    return (feat_out, d_out), res


def kernel(**inputs):
    outs, _ = run(inputs, trace=False)
    return outs


# revision 8
# speedup vs baseline: 1.9244x; 1.9244x over previous
"""GNN message-passing layer (Net3DLayer) on 8 Trainium2 NeuronCores.

Strategy:
  * Host: stable-sort edges by dst; split into 8 contiguous dst ranges
    (6250 nodes each) with roughly E/8 edges. Each core fully owns the
    segment-sum + node update for its node range -> no collectives.
  * Within a core, edges are grouped into 128-node "windows"; each
    window's edge run is padded to a multiple of 128 edges so that the
    device program (one SPMD program for all 8 cores) has a fixed
    structure (window subtile counts are compile-time constants
    computed from the actual index data).
  * Host pre-gathers feat[src], feat[dst] and pre-transposes all edge
    streams to feature-major [128, E_pad] bf16 so the device streams
    them with dense DMA (contraction dim on partitions).
  * Device per 512-edge group: message MLP (bf16 matmuls with f32 PSUM
    accumulate; SiLU computed as sigmoid on ACT + (x+b)*s on GPSIMD so
    the ACT engine only ever needs the Sigmoid table), edge gate
    (matmul + sigmoid + K=1 broadcast matmul + DVE multiply), then per
    128-edge subtile a PE transpose to edge-major and a one-hot scatter
    matmul (m_sum^T[:, win] += m_em^T @ Sel) accumulating in a window
    PSUM tile. When a window completes: node update MLP + residual.
  * d_out = d + message is computed on the host from the returned
    message stream (keeps d at exact f32 and drops the output add from
    the device entirely).
"""

import math
from contextlib import ExitStack

import numpy as np
import ml_dtypes

import concourse.bass as bass
import concourse.tile as tile
from concourse import bacc, mybir
from concourse.bass_utils import run_bass_kernel_spmd

N_NODES = 50000
N_EDGES = 800000
HID = 128
P = 128
N_CORES = 8
NPC = N_NODES // N_CORES  # 6250 nodes per core
W_FULL = (NPC + P - 1) // P  # 49 windows per core (last is 106 wide)

F32 = mybir.dt.float32
BF16 = mybir.dt.bfloat16
AF = mybir.ActivationFunctionType
OP = mybir.AluOpType
BF = ml_dtypes.bfloat16


def _plan(dst_sorted):
    """Window/padding layout shared by all cores (max over cores per slot)."""
    bounds = np.empty((N_CORES, W_FULL + 1), dtype=np.int64)
    for c in range(N_CORES):
        for w in range(W_FULL + 1):
            bounds[c, w] = c * NPC + min(P * w, NPC)
    rs = np.searchsorted(dst_sorted, bounds.reshape(-1)).reshape(N_CORES, W_FULL + 1)
    runlen = rs[:, 1:] - rs[:, :-1]  # [8, 49]
    Lw = np.maximum(1, (-(-runlen // P)).max(axis=0)).astype(np.int64)  # [49]
    S = int(Lw.sum())
    G = (S + 3) // 4
    S_pad = G * 4
    return rs, Lw, S, G, S_pad


def _build_program(G, Lw, S_pad, E_pad, bg_val):
    """Build + compile the SPMD bass program (same for all 8 cores)."""
    nc = bacc.Bacc("TRN2", target_bir_lowering=False, debug=False,
                   num_devices=N_CORES)

    din = {}
    for name, shape, dt in [
        ("fsrcT", [P, E_pad], BF16), ("fdstT", [P, E_pad], BF16),
        ("dT", [P, E_pad], BF16),
        ("dstl", [G * P, 4], F32), ("featTs", [P, W_FULL * P], F32),
        ("w1m", [P, 3 * P], BF16), ("w2m", [P, P], BF16), ("wg", [P, 1], BF16),
        ("w1u", [P, P], BF16), ("w2u", [P, P], BF16),
        ("b1m", [P, 1], F32), ("b2m", [P, 1], F32), ("b1u", [P, 1], F32),
        ("b2u", [P, 1], F32),
        ("iota", [P, P], BF16), ("ident", [P, P], BF16), ("ones", [1, P], BF16),
    ]:
        din[name] = nc.dram_tensor(name, shape, dt, kind="ExternalInput").ap()
    msg_out = nc.dram_tensor("msg_out", [P, E_pad], BF16,
                             kind="ExternalOutput").ap()
    feat_outT = nc.dram_tensor("feat_outT", [P, W_FULL * P], F32,
                               kind="ExternalOutput").ap()

    # subtile -> (window, position) map; dummy tail window index W_FULL
    w_of, pos_of, len_of = [], [], []
    for w in range(W_FULL):
        for t in range(int(Lw[w])):
            w_of.append(w); pos_of.append(t); len_of.append(int(Lw[w]))
    n_tail = S_pad - len(w_of)
    for t in range(n_tail):
        w_of.append(W_FULL); pos_of.append(t); len_of.append(n_tail)

    with tile.TileContext(nc) as tc, ExitStack() as ctx:
        cpool = ctx.enter_context(tc.tile_pool(name="consts", bufs=1))
        spool = ctx.enter_context(tc.tile_pool(name="stream", bufs=4))
        wpool = ctx.enter_context(tc.tile_pool(name="work", bufs=3))
        epool = ctx.enter_context(tc.tile_pool(name="em", bufs=6))
        npool = ctx.enter_context(tc.tile_pool(name="node", bufs=2))
        pmm = ctx.enter_context(tc.tile_pool(name="pmm", bufs=2, space="PSUM"))
        pgl = ctx.enter_context(tc.tile_pool(name="pgl", bufs=2, space="PSUM"))
        ptr = ctx.enter_context(tc.tile_pool(name="ptr", bufs=2, space="PSUM"))
        pwin = ctx.enter_context(tc.tile_pool(name="pwin", bufs=2, space="PSUM"))

        cst = {}
        for name in ["w1m", "w2m", "wg", "w1u", "w2u", "b1m", "b2m", "b1u",
                     "b2u", "iota", "ident", "ones"]:
            ap = din[name]
            t = cpool.tile(list(ap.shape), ap.dtype, name=f"c_{name}", tag=name)
            nc.sync.dma_start(t[:], ap[:])
            cst[name] = t

        win_tiles = {}

        def node_update(w):
            wt = win_tiles.pop(w)
            fTw = npool.tile([P, P], F32, name="fTw", tag="fTw")
            nc.sync.dma_start(fTw[:], din["featTs"][:, w * P:(w + 1) * P])
            xT = npool.tile([P, P], BF16, name="xT", tag="xT")
            nc.vector.tensor_add(xT[:], wt[:], fTw[:])
            ups = ptr.tile([P, P], F32, name="ups", tag="tr")
            nc.tensor.matmul(ups[:], cst["w1u"][:], xT[:], start=True, stop=True,
                             skip_group_check=True)
            su = npool.tile([P, P], BF16, name="su", tag="su")
            nc.scalar.activation(su[:], ups[:], AF.Sigmoid, bias=cst["b1u"][:])
            ulin = npool.tile([P, P], BF16, name="ulin", tag="ulin")
            nc.vector.tensor_scalar(ulin[:], ups[:], cst["b1u"][:], None, OP.add)
            uT = npool.tile([P, P], BF16, name="uT", tag="uT")
            nc.gpsimd.tensor_tensor(uT[:], ulin[:], su[:], op=OP.mult)
            ops_ = ptr.tile([P, P], F32, name="ops", tag="tr")
            nc.tensor.matmul(ops_[:], cst["w2u"][:], uT[:], start=True, stop=True,
                             skip_group_check=True)
            oT = npool.tile([P, P], F32, name="oT", tag="oT")
            nc.vector.scalar_tensor_tensor(oT[:], ops_[:], cst["b2u"][:], fTw[:],
                                           op0=OP.add, op1=OP.add)
            nc.sync.dma_start(feat_outT[:, w * P:(w + 1) * P], oT[:])

        for g in range(G):
            sl = slice(g * 512, (g + 1) * 512)
            fs = spool.tile([P, 512], BF16, name="fs", tag="fs")
            nc.sync.dma_start(fs[:], din["fsrcT"][:, sl])
            fd = spool.tile([P, 512], BF16, name="fd", tag="fd")
            nc.sync.dma_start(fd[:], din["fdstT"][:, sl])
            dd = spool.tile([P, 512], BF16, name="dd", tag="dd")
            nc.sync.dma_start(dd[:], din["dT"][:, sl])
            dl = spool.tile([P, 4], F32, name="dl", tag="dl")
            nc.sync.dma_start(dl[:], din["dstl"][g * P:(g + 1) * P, :])

            hps = pmm.tile([P, 512], F32, name="hps", tag="mm")
            nc.tensor.matmul(hps[:], cst["w1m"][:, 0:P], fs[:],
                             start=True, stop=False, skip_group_check=True)
            nc.tensor.matmul(hps[:], cst["w1m"][:, P:2 * P], fd[:],
                             start=False, stop=False, skip_group_check=True)
            nc.tensor.matmul(hps[:], cst["w1m"][:, 2 * P:3 * P], dd[:],
                             start=False, stop=True, skip_group_check=True)
            sh = wpool.tile([P, 512], BF16, name="sh", tag="sh")
            nc.scalar.activation(sh[:], hps[:], AF.Sigmoid, bias=cst["b1m"][:])
            hlin = wpool.tile([P, 512], BF16, name="hlin", tag="hlin")
            nc.vector.tensor_scalar(hlin[:], hps[:], cst["b1m"][:], None, OP.add)
            hT = wpool.tile([P, 512], BF16, name="hT", tag="hT")
            nc.gpsimd.tensor_tensor(hT[:], hlin[:], sh[:], op=OP.mult)

            mps = pmm.tile([P, 512], F32, name="mps", tag="mm")
            nc.tensor.matmul(mps[:], cst["w2m"][:], hT[:], start=True, stop=True,
                             skip_group_check=True)
            sm = wpool.tile([P, 512], BF16, name="sm", tag="sm")
            nc.scalar.activation(sm[:], mps[:], AF.Sigmoid, bias=cst["b2m"][:])
            mlin = wpool.tile([P, 512], BF16, name="mlin", tag="mlin")
            nc.vector.tensor_scalar(mlin[:], mps[:], cst["b2m"][:], None, OP.add)
            msgT = wpool.tile([P, 512], BF16, name="msgT", tag="msgT")
            nc.gpsimd.tensor_tensor(msgT[:], mlin[:], sm[:], op=OP.mult)
            nc.sync.dma_start(msg_out[:, sl], msgT[:])

            # edge-major gate logits for the 4 subtiles: [128 edges, 4]
            glps = pgl.tile([P, 4], F32, name="glps", tag="gl")
            for j in range(4):
                nc.tensor.matmul(glps[:, j:j + 1], msgT[:, j * P:(j + 1) * P],
                                 cst["wg"][:], start=True, stop=True,
                                 skip_group_check=True)
            gE = wpool.tile([P, 4], F32, name="gE", tag="gE")
            nc.scalar.activation(gE[:], glps[:], AF.Sigmoid, bias=float(bg_val))

            for j in range(4):
                s = g * 4 + j
                w, pos, lw = w_of[s], pos_of[s], len_of[s]
                tps = ptr.tile([P, P], BF16, name="tps", tag="tr")
                nc.tensor.transpose(tps[:], msgT[:, j * P:(j + 1) * P],
                                    cst["ident"][:])
                mem = epool.tile([P, P], BF16, name="mem", tag="mem")
                if j % 2 == 0:
                    nc.scalar.copy(mem[:], tps[:])
                else:
                    nc.vector.tensor_copy(mem[:], tps[:])
                # Sel[e, n] = (iota[n] == dst_local[e]) * gate[e]
                sel = epool.tile([P, P], BF16, name="sel", tag="sel")
                nc.vector.tensor_scalar(sel[:], cst["iota"][:], dl[:, j:j + 1],
                                        gE[:, j:j + 1], OP.is_equal,
                                        op1=OP.mult)
                if pos == 0:
                    win_tiles[w] = pwin.tile([P, P], F32, name=f"win{w}",
                                             tag="win")
                nc.tensor.matmul(win_tiles[w][:], mem[:], sel[:],
                                 start=(pos == 0), stop=(pos == lw - 1),
                                 skip_group_check=True)
                if pos == lw - 1:
                    if w < W_FULL:
                        node_update(w)
                    else:
                        win_tiles.pop(w)

    nc.compile()
    return nc


def _prepare(feat, d, src, dst, W1m, b1m, W2m, b2m, Wg, bg, W1u, b1u, W2u, b2u):
    feat = np.ascontiguousarray(np.asarray(feat, dtype=np.float32))
    d = np.ascontiguousarray(np.asarray(d, dtype=np.float32))
    src = np.asarray(src).astype(np.int64).ravel()
    dst = np.asarray(dst).astype(np.int64).ravel()
    E = dst.shape[0]

    order = np.argsort(dst, kind="stable")
    dst_s = dst[order]
    rs, Lw, S, G, S_pad = _plan(dst_s)
    E_pad = S_pad * P

    woff = np.concatenate([[0], np.cumsum(Lw)]).astype(np.int64)

    # per-core original-edge-index map (-1 = padding) and local dst offsets
    map_pad = np.full((N_CORES, E_pad), -1, dtype=np.int64)
    dstl_pad = np.full((N_CORES, E_pad), -5.0, dtype=np.float32)
    for c in range(N_CORES):
        for w in range(W_FULL):
            a, b = int(rs[c, w]), int(rs[c, w + 1])
            n = b - a
            if n == 0:
                continue
            base = int(woff[w]) * P
            map_pad[c, base:base + n] = order[a:b]
            dstl_pad[c, base:base + n] = (
                dst_s[a:b] - (c * NPC + P * w)).astype(np.float32)

    featT_padded = np.zeros((P, N_CORES * NPC + P), dtype=np.float32)
    featT_padded[:, :N_NODES] = feat.T

    featb = feat.astype(BF)
    d_extb = np.vstack([d.astype(BF), np.zeros((1, HID), dtype=BF)])

    in_maps = []
    consts = {
        "w1m": np.ascontiguousarray(
            np.concatenate([W1m[0:P], W1m[P:2 * P], W1m[2 * P:3 * P]], axis=1)
        ).astype(BF),
        "w2m": np.ascontiguousarray(np.asarray(W2m, np.float32)).astype(BF),
        "wg": np.ascontiguousarray(
            np.asarray(Wg, np.float32).reshape(P, 1)).astype(BF),
        "w1u": np.ascontiguousarray(np.asarray(W1u, np.float32)).astype(BF),
        "w2u": np.ascontiguousarray(np.asarray(W2u, np.float32)).astype(BF),
        "b1m": np.asarray(b1m, np.float32).reshape(P, 1).copy(),
        "b2m": np.asarray(b2m, np.float32).reshape(P, 1).copy(),
        "b1u": np.asarray(b1u, np.float32).reshape(P, 1).copy(),
        "b2u": np.asarray(b2u, np.float32).reshape(P, 1).copy(),
        "iota": np.broadcast_to(np.arange(P, dtype=np.float32),
                                (P, P)).astype(BF),
        "ident": np.eye(P, dtype=np.float32).astype(BF),
        "ones": np.ones((1, P), dtype=BF),
    }
    for c in range(N_CORES):
        idx = map_pad[c]
        valid = idx >= 0
        safe = np.where(valid, idx, 0)
        src_idx = np.where(valid, src[safe], 0)
        dst_idx = np.where(valid, dst[safe], 0)
        didx = np.where(valid, idx, E)
        m = {
            "fsrcT": np.ascontiguousarray(featb[src_idx].T),
            "fdstT": np.ascontiguousarray(featb[dst_idx].T),
            "dT": np.ascontiguousarray(d_extb[didx].T),
            "dstl": np.ascontiguousarray(
                dstl_pad[c].reshape(G, 4, P).transpose(0, 2, 1).reshape(G * P, 4)
            ),
            "featTs": np.ascontiguousarray(
                featT_padded[:, c * NPC:c * NPC + W_FULL * P]),
        }
        m.update(consts)
        in_maps.append(m)

    bg_val = float(np.asarray(bg, np.float32).ravel()[0])
    meta = dict(E=E, G=G, Lw=Lw, S_pad=S_pad, E_pad=E_pad, map_pad=map_pad,
                bg_val=bg_val, d=d)
    return in_maps, meta


def _assemble(results, meta):
    E = meta["E"]
    feat_out = np.empty((N_NODES, HID), dtype=np.float32)
    d_out = np.array(meta["d"], dtype=np.float32, copy=True)
    for c in range(N_CORES):
        fo = results[c]["feat_outT"]  # [128, W_FULL*128]
        feat_out[c * NPC:(c + 1) * NPC] = fo.T[:NPC]
        mo = results[c]["msg_out"]  # [128, E_pad] bf16
        idx = meta["map_pad"][c]
        v = idx >= 0
        d_out[idx[v]] += mo.T[v].astype(np.float32)
    return feat_out, d_out


def run(inputs, trace=False, tmpdir=None):
    in_maps, meta = _prepare(**inputs)
    nc = _build_program(meta["G"], meta["Lw"], meta["S_pad"], meta["E_pad"],
                        meta["bg_val"])
    res = run_bass_kernel_spmd(nc, in_maps, core_ids=list(range(N_CORES)),
                               trace=trace, tmpdir=tmpdir)
    feat_out, d_out = _assemble(res.results, meta)
    return (feat_out, d_out), res


def kernel(**inputs):
    outs, _ = run(inputs, trace=False)
    return outs


# revision 9
# speedup vs baseline: 2.9884x; 1.5529x over previous
"""GNN message-passing layer (Net3DLayer) on 8 Trainium2 NeuronCores.

Strategy:
  * Host: stable-sort edges by dst; split into 8 contiguous dst ranges
    (6250 nodes each) with roughly E/8 edges. Each core fully owns the
    segment-sum + node update for its node range -> no collectives.
  * Within a core, edges are grouped into 128-node "windows"; each
    window's edge run is padded to a multiple of 128 edges so that the
    device program (one SPMD program for all 8 cores) has a fixed
    structure (window subtile counts are compile-time constants
    computed from the actual index data).
  * Host pre-gathers feat[src], feat[dst] and pre-transposes all edge
    streams to feature-major [128, E_pad] bf16 so the device streams
    them with dense DMA (contraction dim on partitions).
  * Device per 512-edge group: message MLP (bf16 matmuls with f32 PSUM
    accumulate; SiLU computed as sigmoid on ACT + (x+b)*s on GPSIMD so
    the ACT engine only ever needs the Sigmoid table), edge gate
    (matmul + sigmoid + K=1 broadcast matmul + DVE multiply), then per
    128-edge subtile a PE transpose to edge-major and a one-hot scatter
    matmul (m_sum^T[:, win] += m_em^T @ Sel) accumulating in a window
    PSUM tile. When a window completes: node update MLP + residual.
  * d_out = d + message is computed on the host from the returned
    message stream (keeps d at exact f32 and drops the output add from
    the device entirely).
"""

import math
from contextlib import ExitStack

import numpy as np
import ml_dtypes

import concourse.bass as bass
import concourse.tile as tile
from concourse import bacc, mybir
from concourse.bass_utils import run_bass_kernel_spmd

N_NODES = 50000
N_EDGES = 800000
HID = 128
P = 128
N_CORES = 8
NPC = N_NODES // N_CORES  # 6250 nodes per core
W_FULL = (NPC + P - 1) // P  # 49 windows per core (last is 106 wide)

F32 = mybir.dt.float32
BF16 = mybir.dt.bfloat16
AF = mybir.ActivationFunctionType
OP = mybir.AluOpType
BF = ml_dtypes.bfloat16


def _plan(dst_sorted):
    """Window/padding layout shared by all cores (max over cores per slot)."""
    bounds = np.empty((N_CORES, W_FULL + 1), dtype=np.int64)
    for c in range(N_CORES):
        for w in range(W_FULL + 1):
            bounds[c, w] = c * NPC + min(P * w, NPC)
    rs = np.searchsorted(dst_sorted, bounds.reshape(-1)).reshape(N_CORES, W_FULL + 1)
    runlen = rs[:, 1:] - rs[:, :-1]  # [8, 49]
    Lw = np.maximum(1, (-(-runlen // P)).max(axis=0)).astype(np.int64)  # [49]
    S = int(Lw.sum())
    G = (S + 3) // 4
    S_pad = G * 4
    return rs, Lw, S, G, S_pad


def _build_program(G, Lw, S_pad, E_pad, bg_val):
    """Build + compile the SPMD bass program (same for all 8 cores)."""
    nc = bacc.Bacc("TRN2", target_bir_lowering=False, debug=False,
                   num_devices=N_CORES)

    din = {}
    for name, shape, dt in [
        ("xin", [P, 3 * E_pad], BF16),
        ("dstl", [G * P, 4], F32), ("featTs", [P, W_FULL * P], F32),
        ("w1m", [P, 3 * P], BF16), ("w2m", [P, P], BF16), ("wg", [P, 1], BF16),
        ("w1u", [P, P], BF16), ("w2u", [P, P], BF16),
        ("b1m", [P, 1], F32), ("b2m", [P, 1], F32), ("b1u", [P, 1], F32),
        ("b2u", [P, 1], F32),
        ("iota", [P, P], BF16), ("ident", [P, P], BF16), ("ones", [1, P], BF16),
    ]:
        din[name] = nc.dram_tensor(name, shape, dt, kind="ExternalInput").ap()
    msg_out = nc.dram_tensor("msg_out", [P, E_pad], BF16,
                             kind="ExternalOutput").ap()
    feat_outT = nc.dram_tensor("feat_outT", [P, W_FULL * P], F32,
                               kind="ExternalOutput").ap()

    # subtile -> (window, position) map; dummy tail window index W_FULL
    w_of, pos_of, len_of = [], [], []
    for w in range(W_FULL):
        for t in range(int(Lw[w])):
            w_of.append(w); pos_of.append(t); len_of.append(int(Lw[w]))
    n_tail = S_pad - len(w_of)
    for t in range(n_tail):
        w_of.append(W_FULL); pos_of.append(t); len_of.append(n_tail)

    with tile.TileContext(nc) as tc, ExitStack() as ctx:
        cpool = ctx.enter_context(tc.tile_pool(name="consts", bufs=1))
        spool = ctx.enter_context(tc.tile_pool(name="stream", bufs=4))
        wpool = ctx.enter_context(tc.tile_pool(name="work", bufs=3))
        epool = ctx.enter_context(tc.tile_pool(name="em", bufs=6))
        npool = ctx.enter_context(tc.tile_pool(name="node", bufs=2))
        pmm = ctx.enter_context(tc.tile_pool(name="pmm", bufs=2, space="PSUM"))
        pgl = ctx.enter_context(tc.tile_pool(name="pgl", bufs=2, space="PSUM"))
        ptr = ctx.enter_context(tc.tile_pool(name="ptr", bufs=2, space="PSUM"))
        pwin = ctx.enter_context(tc.tile_pool(name="pwin", bufs=2, space="PSUM"))

        cst = {}
        for name in ["w1m", "w2m", "wg", "w1u", "w2u", "b1m", "b2m", "b1u",
                     "b2u", "iota", "ident", "ones"]:
            ap = din[name]
            t = cpool.tile(list(ap.shape), ap.dtype, name=f"c_{name}", tag=name)
            nc.sync.dma_start(t[:], ap[:])
            cst[name] = t

        win_tiles = {}

        def node_update(w):
            wt = win_tiles.pop(w)
            fTw = npool.tile([P, P], F32, name="fTw", tag="fTw")
            nc.sync.dma_start(fTw[:], din["featTs"][:, w * P:(w + 1) * P])
            xT = npool.tile([P, P], BF16, name="xT", tag="xT")
            nc.vector.tensor_add(xT[:], wt[:], fTw[:])
            ups = ptr.tile([P, P], F32, name="ups", tag="tr")
            nc.tensor.matmul(ups[:], cst["w1u"][:], xT[:], start=True, stop=True,
                             skip_group_check=True)
            uT = npool.tile([P, P], BF16, name="uT", tag="uT")
            nc.scalar.activation(uT[:], ups[:], AF.Silu, bias=cst["b1u"][:])
            ops_ = ptr.tile([P, P], F32, name="ops", tag="tr")
            nc.tensor.matmul(ops_[:], cst["w2u"][:], uT[:], start=True, stop=True,
                             skip_group_check=True)
            oT = npool.tile([P, P], F32, name="oT", tag="oT")
            nc.vector.scalar_tensor_tensor(oT[:], ops_[:], cst["b2u"][:], fTw[:],
                                           op0=OP.add, op1=OP.add)
            nc.sync.dma_start(feat_outT[:, w * P:(w + 1) * P], oT[:])

        for g in range(G):
            sl = slice(g * 512, (g + 1) * 512)
            xg = spool.tile([P, 1536], BF16, name="xg", tag="xg")
            nc.sync.dma_start(xg[:], din["xin"][:, g * 1536:(g + 1) * 1536])
            dl = spool.tile([P, 4], F32, name="dl", tag="dl")
            nc.sync.dma_start(dl[:], din["dstl"][g * P:(g + 1) * P, :])

            hps = pmm.tile([P, 512], F32, name="hps", tag="mm")
            nc.tensor.matmul(hps[:], cst["w1m"][:, 0:P], xg[:, 0:512],
                             start=True, stop=False, skip_group_check=True)
            nc.tensor.matmul(hps[:], cst["w1m"][:, P:2 * P], xg[:, 512:1024],
                             start=False, stop=False, skip_group_check=True)
            nc.tensor.matmul(hps[:], cst["w1m"][:, 2 * P:3 * P], xg[:, 1024:1536],
                             start=False, stop=True, skip_group_check=True)
            hT = wpool.tile([P, 512], BF16, name="hT", tag="hT")
            nc.scalar.activation(hT[:], hps[:], AF.Silu, bias=cst["b1m"][:])

            mps = pmm.tile([P, 512], F32, name="mps", tag="mm")
            nc.tensor.matmul(mps[:], cst["w2m"][:], hT[:], start=True, stop=True,
                             skip_group_check=True)
            msgT = wpool.tile([P, 512], BF16, name="msgT", tag="msgT")
            nc.scalar.activation(msgT[:], mps[:], AF.Silu, bias=cst["b2m"][:])
            nc.sync.dma_start(msg_out[:, sl], msgT[:])

            # edge-major gate logits for the 4 subtiles: [128 edges, 4]
            # gate = sigmoid(z+bg) = (tanh((z+bg)/2) + 1)/2  (tanh shares the
            # silu ACT table set, so no table reloads)
            glps = pgl.tile([P, 4], F32, name="glps", tag="gl")
            for j in range(4):
                nc.tensor.matmul(glps[:, j:j + 1], msgT[:, j * P:(j + 1) * P],
                                 cst["wg"][:], start=True, stop=True,
                                 skip_group_check=True)
            gt = wpool.tile([P, 4], F32, name="gt", tag="gt")
            nc.scalar.activation(gt[:], glps[:], AF.Tanh, bias=float(bg_val) / 2,
                                 scale=0.5)
            gE = wpool.tile([P, 4], F32, name="gE", tag="gE")
            nc.vector.tensor_scalar(gE[:], gt[:], 1.0, 0.5, OP.add, op1=OP.mult)

            for j in range(4):
                s = g * 4 + j
                w, pos, lw = w_of[s], pos_of[s], len_of[s]
                tps = ptr.tile([P, P], BF16, name="tps", tag="tr")
                nc.tensor.transpose(tps[:], msgT[:, j * P:(j + 1) * P],
                                    cst["ident"][:])
                # gated edge-major message: mem[e, :] = tps[e, :] * gate[e]
                mem = epool.tile([P, P], BF16, name="mem", tag="mem")
                if j % 2 == 0:
                    nc.scalar.activation(mem[:], tps[:], AF.Copy, bias=0.0,
                                         scale=gE[:, j:j + 1])
                else:
                    nc.vector.tensor_scalar(mem[:], tps[:], gE[:, j:j + 1],
                                            None, OP.mult)
                sel = epool.tile([P, P], BF16, name="sel", tag="sel")
                nc.vector.tensor_scalar(sel[:], cst["iota"][:], dl[:, j:j + 1],
                                        None, OP.is_equal)
                if pos == 0:
                    win_tiles[w] = pwin.tile([P, P], F32, name=f"win{w}",
                                             tag="win")
                nc.tensor.matmul(win_tiles[w][:], mem[:], sel[:],
                                 start=(pos == 0), stop=(pos == lw - 1),
                                 skip_group_check=True)
                if pos == lw - 1:
                    if w < W_FULL:
                        node_update(w)
                    else:
                        win_tiles.pop(w)

    nc.compile()
    return nc


def _prepare(feat, d, src, dst, W1m, b1m, W2m, b2m, Wg, bg, W1u, b1u, W2u, b2u):
    feat = np.ascontiguousarray(np.asarray(feat, dtype=np.float32))
    d = np.ascontiguousarray(np.asarray(d, dtype=np.float32))
    src = np.asarray(src).astype(np.int64).ravel()
    dst = np.asarray(dst).astype(np.int64).ravel()
    E = dst.shape[0]

    order = np.argsort(dst, kind="stable")
    dst_s = dst[order]
    rs, Lw, S, G, S_pad = _plan(dst_s)
    E_pad = S_pad * P

    woff = np.concatenate([[0], np.cumsum(Lw)]).astype(np.int64)

    # per-core original-edge-index map (-1 = padding) and local dst offsets
    map_pad = np.full((N_CORES, E_pad), -1, dtype=np.int64)
    dstl_pad = np.full((N_CORES, E_pad), -5.0, dtype=np.float32)
    for c in range(N_CORES):
        for w in range(W_FULL):
            a, b = int(rs[c, w]), int(rs[c, w + 1])
            n = b - a
            if n == 0:
                continue
            base = int(woff[w]) * P
            map_pad[c, base:base + n] = order[a:b]
            dstl_pad[c, base:base + n] = (
                dst_s[a:b] - (c * NPC + P * w)).astype(np.float32)

    featT_padded = np.zeros((P, N_CORES * NPC + P), dtype=np.float32)
    featT_padded[:, :N_NODES] = feat.T

    featb = feat.astype(BF)
    d_extb = np.vstack([d.astype(BF), np.zeros((1, HID), dtype=BF)])

    in_maps = []
    consts = {
        "w1m": np.ascontiguousarray(
            np.concatenate([W1m[0:P], W1m[P:2 * P], W1m[2 * P:3 * P]], axis=1)
        ).astype(BF),
        "w2m": np.ascontiguousarray(np.asarray(W2m, np.float32)).astype(BF),
        "wg": np.ascontiguousarray(
            np.asarray(Wg, np.float32).reshape(P, 1)).astype(BF),
        "w1u": np.ascontiguousarray(np.asarray(W1u, np.float32)).astype(BF),
        "w2u": np.ascontiguousarray(np.asarray(W2u, np.float32)).astype(BF),
        "b1m": np.asarray(b1m, np.float32).reshape(P, 1).copy(),
        "b2m": np.asarray(b2m, np.float32).reshape(P, 1).copy(),
        "b1u": np.asarray(b1u, np.float32).reshape(P, 1).copy(),
        "b2u": np.asarray(b2u, np.float32).reshape(P, 1).copy(),
        "iota": np.broadcast_to(np.arange(P, dtype=np.float32),
                                (P, P)).astype(BF),
        "ident": np.eye(P, dtype=np.float32).astype(BF),
        "ones": np.ones((1, P), dtype=BF),
    }
    for c in range(N_CORES):
        idx = map_pad[c]
        valid = idx >= 0
        safe = np.where(valid, idx, 0)
        src_idx = np.where(valid, src[safe], 0)
        dst_idx = np.where(valid, dst[safe], 0)
        didx = np.where(valid, idx, E)
        xin = np.empty((P, G, 3, 512), dtype=BF)
        xin[:, :, 0, :] = featb[src_idx].T.reshape(P, G, 512)
        xin[:, :, 1, :] = featb[dst_idx].T.reshape(P, G, 512)
        xin[:, :, 2, :] = d_extb[didx].T.reshape(P, G, 512)
        m = {
            "xin": np.ascontiguousarray(xin.reshape(P, 3 * E_pad)),
            "dstl": np.ascontiguousarray(
                dstl_pad[c].reshape(G, 4, P).transpose(0, 2, 1).reshape(G * P, 4)
            ),
            "featTs": np.ascontiguousarray(
                featT_padded[:, c * NPC:c * NPC + W_FULL * P]),
        }
        m.update(consts)
        in_maps.append(m)

    bg_val = float(np.asarray(bg, np.float32).ravel()[0])
    meta = dict(E=E, G=G, Lw=Lw, S_pad=S_pad, E_pad=E_pad, map_pad=map_pad,
                bg_val=bg_val, d=d)
    return in_maps, meta


def _assemble(results, meta):
    E = meta["E"]
    feat_out = np.empty((N_NODES, HID), dtype=np.float32)
    d_out = np.array(meta["d"], dtype=np.float32, copy=True)
    for c in range(N_CORES):
        fo = results[c]["feat_outT"]  # [128, W_FULL*128]
        feat_out[c * NPC:(c + 1) * NPC] = fo.T[:NPC]
        mo = results[c]["msg_out"]  # [128, E_pad] bf16
        idx = meta["map_pad"][c]
        v = idx >= 0
        d_out[idx[v]] += mo.T[v].astype(np.float32)
    return feat_out, d_out


def run(inputs, trace=False, tmpdir=None):
    in_maps, meta = _prepare(**inputs)
    nc = _build_program(meta["G"], meta["Lw"], meta["S_pad"], meta["E_pad"],
                        meta["bg_val"])
    res = run_bass_kernel_spmd(nc, in_maps, core_ids=list(range(N_CORES)),
                               trace=trace, tmpdir=tmpdir)
    feat_out, d_out = _assemble(res.results, meta)
    return (feat_out, d_out), res


def kernel(**inputs):
    outs, _ = run(inputs, trace=False)
    return outs
